# revision 16
# baseline (speedup 1.0000x reference)
"""Trainium2 Bass kernel for symmetric self-attention with Laplacian regularizer.

Shapes (hardcoded): B=4, S=1024, D=768, H=12, HD=64.
Sharding: 8 cores = (batch b = c//2) x (head-half g = c%2, 6 heads each).

Key algebra (per batch, head), exploiting that scores are SYMMETRIC (k==q):
  s = q q^T / 8;  X = exp(s)  (safe without max-shift; |s| small)
  Z_i = rowsum(X); w = 1/Z;  probs = 0.5 (diag(w) X + X diag(w))
  dvec = 0.5 (1 + X w);  g = 1/(1 + X w)
  ctx_raw = diag(w)(Xv) + X(w*v);  ctx = 0.5*ctx_raw  (0.5 folded into Wo)
  V = v diag(1/vnorm) never materialized:
  tr  = |3072 - sum g_s * ctx_raw[s,d] * v[s,d] / vn2_d| / 3072
  reg = (sum_de G_de^2 / (vn2_d vn2_e) - 3072) / 3072,   G = v^T v

SPMD trick: per-core row-halves are expressed by ROTATING hsT's s-columns by
g*512 on the host (all s-dependent math is permutation-equivariant); Wq's
output columns are rotated by g*384 so qT always uses columns 0:384. Host
un-rotates the outputs.
"""

from contextlib import ExitStack

import numpy as np

import concourse.bass as bass
import concourse.bacc as bacc
import concourse.tile as tile
from concourse import mybir
from concourse.bass_utils import run_bass_kernel_spmd
import os
KSTAGE = int(os.environ.get("KSTAGE", "3"))

P = 128
B, S, D, H = 4, 1024, 768, 12
HD = 64
HPC = 6          # heads per core
DG = HPC * HD    # 384 head-dims per core
KD = D // P      # 6 contraction chunks over D
ST = S // P      # 8 sequence tiles
KG = DG // P     # 3 chunks over DG
AOp = mybir.AluOpType

F32 = mybir.dt.float32
F32R = mybir.dt.float32r

USE_R_PROJ = False
USE_R_SCORES = False
USE_R_CTX = False
USE_R_WO = False


def _r(ap, enable):
    return ap.bitcast(F32R) if enable else ap


def build_program():
    nc = bacc.Bacc("TRN2", target_bir_lowering=False)

    hsT = nc.dram_tensor("hsT", [D, S], F32, kind="ExternalInput").ap()
    wqf = nc.dram_tensor("wqf", [D, D], F32, kind="ExternalInput").ap()
    wvh = nc.dram_tensor("wvh", [D, DG], F32, kind="ExternalInput").ap()
    woh = nc.dram_tensor("woh", [DG, D], F32, kind="ExternalInput").ap()
    bqc = nc.dram_tensor("bqc", [P, KG], F32, kind="ExternalInput").ap()
    bvr = nc.dram_tensor("bvr", [1, DG], F32, kind="ExternalInput").ap()
    bvc = nc.dram_tensor("bvc", [P, KG], F32, kind="ExternalInput").ap()
    ident = nc.dram_tensor("ident", [P, P], F32, kind="ExternalInput").ap()

    mq_out = nc.dram_tensor("mq_out", [S // 2, D], F32, kind="ExternalOutput").ap()
    att_out = nc.dram_tensor("att_out", [S, D], F32, kind="ExternalOutput").ap()
    red_out = nc.dram_tensor("red_out", [P, 2], F32, kind="ExternalOutput").ap()

    def bc_dram_row(row_ap, parts):
        # DRAM row [1, N] (or [N]) replicated across `parts` partitions for DMA
        a = row_ap
        return bass.AP(tensor=a.tensor, offset=a.offset, ap=[[0, parts], list(a.ap[-1])])

    with tile.TileContext(nc) as tc, ExitStack() as ctx:
        sing = ctx.enter_context(tc.tile_pool(name="sing", bufs=1))
        work = ctx.enter_context(tc.tile_pool(name="work", bufs=2))
        dram = ctx.enter_context(tc.tile_pool(name="dram", bufs=1, space="DRAM"))
        psS = ctx.enter_context(tc.tile_pool(name="psS", bufs=2, space="PSUM"))
        psC = ctx.enter_context(tc.tile_pool(name="psC", bufs=2, space="PSUM"))
        psG = ctx.enter_context(tc.tile_pool(name="psG", bufs=1, space="PSUM"))
        psX = ctx.enter_context(tc.tile_pool(name="psX", bufs=1, space="PSUM"))

        # ---- constants / weights ----
        hsT_sb = sing.tile([P, KD, S], F32)
        nc.sync.dma_start(hsT_sb, hsT.rearrange("(c p) s -> p c s", p=P))
        # wqf shares its slot with X_sb (tag XA): wqf is dead once projections
        # finish, exactly when X is first written
        wqf_sb = sing.tile([P, KD, D], F32, tag="XA")
        nc.sync.dma_start(wqf_sb, wqf.rearrange("(c p) n -> p c n", p=P))
        wvh_sb = sing.tile([P, KD, DG], F32)
        nc.sync.dma_start(wvh_sb, wvh.rearrange("(c p) n -> p c n", p=P))
        woh_sb = sing.tile([P, KG, D], F32)
        nc.sync.dma_start(woh_sb, woh.rearrange("(c p) n -> p c n", p=P))
        bqc_sb = sing.tile([P, KG], F32)
        nc.sync.dma_start(bqc_sb, bqc)
        bvr_sb = sing.tile([P, DG], F32)
        nc.sync.dma_start(bvr_sb, bc_dram_row(bvr, P))
        bvc_sb = sing.tile([P, KG], F32)
        nc.sync.dma_start(bvc_sb, bvc)
        ident_sb = sing.tile([P, P], F32)
        nc.sync.dma_start(ident_sb, ident)

        qT_sb = sing.tile([P, KG, S], F32)
        vT_sb = sing.tile([P, KG, S], F32)
        v_sb = sing.tile([P, ST, DG], F32)
        ctx_sb = sing.tile([P, ST, DG], F32)    # ctx_raw natural [s, d]
        ctxT_pack = sing.tile([P, KG, S], F32)  # ctx_raw^T for the Wo matmul
        X_sb = sing.tile([P, ST, S], F32, tag="XA")
        vn2 = sing.tile([P, KG], F32)
        ivv2 = sing.tile([P, KG], F32)
        ivv2_bc = sing.tile([P, DG], F32)
        ivv2_h = sing.tile([HD, HPC], F32)      # per-head columnar, base partition 0
        tracc = sing.tile([P, 1], F32)
        regacc = sing.tile([P, 1], F32)
        nc.vector.memset(tracc, 0.0)
        nc.vector.memset(regacc, 0.0)

        # PE touch-matmuls: absorb each input-DMA's queue semaphore on the PE
        # one at a time (the LDWEIGHTS uop allows only a single sync wait, so
        # a real matmul must not be the first PE consumer of TWO fresh DMAs)
        touches = (hsT_sb[0:1, 0, 0:1], wqf_sb[0:1, 0, 0:1],
                   wvh_sb[0:1, 0, 0:1], woh_sb[0:1, 0, 0:1])
        pj = psG.tile([1, 1], F32, tag="g")
        for i, tch in enumerate(touches):
            nc.tensor.matmul(pj, tch, tch, start=(i == 0), stop=(i == len(touches) - 1))

        # ---- projections ----
        # mixed_q rows 0:512 of (rotated) s: out = hs @ Wq_rot (bias on host)
        for st in range(4):
            ps = psS.tile([P, S], F32, tag="big")
            for n2 in range(2):
                n0, n1 = n2 * 512, min(D, (n2 + 1) * 512)
                for k in range(KD):
                    nc.tensor.matmul(
                        ps[:, n0:n1],
                        _r(hsT_sb[:, k, st * P : (st + 1) * P], USE_R_PROJ),
                        _r(wqf_sb[:, k, n0:n1], USE_R_PROJ),
                        start=(k == 0), stop=(k == KD - 1),
                    )
            stg = work.tile([P, D], F32, tag="ostage")
            nc.vector.tensor_copy(stg, ps[:, :D])
            nc.gpsimd.dma_start(mq_out[st * P : (st + 1) * P, :], stg)

        # qT (+bq per-partition): lhsT = Wq_rot cols 0:384
        for mt in range(KG):
            ps = psS.tile([P, S], F32, tag="big")
            for n2 in range(2):
                n0, n1 = n2 * 512, (n2 + 1) * 512
                for k in range(KD):
                    nc.tensor.matmul(
                        ps[:, n0:n1],
                        _r(wqf_sb[:, k, mt * P : (mt + 1) * P], USE_R_PROJ),
                        _r(hsT_sb[:, k, n0:n1], USE_R_PROJ),
                        start=(k == 0), stop=(k == KD - 1),
                    )
            nc.vector.tensor_scalar_add(qT_sb[:, mt, :], ps, bqc_sb[:, mt : mt + 1])

        # vT (+bv per-partition)
        for mt in range(KG):
            ps = psS.tile([P, S], F32, tag="big")
            for n2 in range(2):
                n0, n1 = n2 * 512, (n2 + 1) * 512
                for k in range(KD):
                    nc.tensor.matmul(
                        ps[:, n0:n1],
                        _r(wvh_sb[:, k, mt * P : (mt + 1) * P], USE_R_PROJ),
                        _r(hsT_sb[:, k, n0:n1], USE_R_PROJ),
                        start=(k == 0), stop=(k == KD - 1),
                    )
            nc.vector.tensor_scalar_add(vT_sb[:, mt, :], ps, bvc_sb[:, mt : mt + 1])

        # v natural (+bv broadcast row)
        for st in range(ST):
            ps = psS.tile([P, S], F32, tag="big")
            for k in range(KD):
                nc.tensor.matmul(
                    ps[:, :DG],
                    _r(hsT_sb[:, k, st * P : (st + 1) * P], USE_R_PROJ),
                    _r(wvh_sb[:, k, :], USE_R_PROJ),
                    start=(k == 0), stop=(k == KD - 1),
                )
            nc.vector.tensor_tensor(v_sb[:, st, :], ps[:, :DG], bvr_sb, AOp.add)

        # vnorm^2 and reciprocals
        for mt in range(KG):
            scr = work.tile([P, S], F32, tag="scrbig")
            nc.vector.tensor_tensor(scr, vT_sb[:, mt, :], vT_sb[:, mt, :], AOp.mult)
            nc.vector.tensor_reduce(vn2[:, mt : mt + 1], scr, mybir.AxisListType.X, AOp.add)
        nc.vector.reciprocal(ivv2, vn2)

        # roundtrip ivv2 [P,3] -> flat row, broadcast to [P, DG]
        ivv2_dram = dram.tile([KG, P], F32)
        nc.gpsimd.dma_start(ivv2_dram.rearrange("t p -> p t"), ivv2)
        nc.sync.dma_start(ivv2_bc, bc_dram_row(ivv2_dram.rearrange("t p -> (t p)")[None, :], P))
        nc.sync.dma_start(ivv2_h, ivv2_dram.rearrange("t (c p) -> p (t c)", p=HD, c=2))

        # ---- per-head attention ----
        # R = [v_h | w*v_h | w | zeros] (rhs of the ctx matmul, N=256 for the
        # f32r fast path; the Xw column rides along at index 128)
        Rn = sing.tile([P, ST, 2 * P], F32)
        nc.vector.memset(Rn, 0.0)

        for h in range(HPC if KSTAGE >= 2 else 0):
            ht, hoff = h // 2, (h % 2) * HD
            hs0 = h * HD

            # scores -> X=exp(s/8) with fused rowsum Z
            Zc = work.tile([P, ST], F32, tag="zc")
            for mt in range(ST):
                ps = psS.tile([P, S], F32, tag="big")
                for n2 in range(2):
                    nc.tensor.matmul(
                        ps[:, n2 * 512 : (n2 + 1) * 512],
                        _r(qT_sb[hoff : hoff + HD, ht, mt * P : (mt + 1) * P], USE_R_SCORES),
                        _r(qT_sb[hoff : hoff + HD, ht, n2 * 512 : (n2 + 1) * 512], USE_R_SCORES),
                        start=True, stop=True,
                    )
                nc.scalar.activation(
                    X_sb[:, mt, :], ps,
                    mybir.ActivationFunctionType.Exp,
                    scale=0.125,
                    accum_out=Zc[:, mt : mt + 1],
                )

            wc = work.tile([P, ST], F32, tag="wc")
            nc.vector.reciprocal(wc, Zc)

            for st in range(ST):
                nc.vector.tensor_copy(Rn[:, st, 0:HD], v_sb[:, st, hs0 : hs0 + HD])
                nc.vector.tensor_scalar_mul(
                    Rn[:, st, HD : 2 * HD], v_sb[:, st, hs0 : hs0 + HD], wc[:, st : st + 1]
                )
                nc.vector.tensor_copy(Rn[:, st, 2 * HD : 2 * HD + 1], wc[:, st : st + 1])

            # ctx matmul (natural orientation): out[s, 0:64]=Xv, [64:128]=Xwv,
            # [128]=Xw.  lhsT = X tiles (symmetric), rhs = Rn.
            rowred8 = work.tile([P, ST], F32, tag="rowred")
            gcol8 = work.tile([P, ST], F32, tag="gcol")
            for mt in range(ST):
                psc = psC.tile([P, 512], F32, tag="ctx")
                for k in range(ST):
                    nc.tensor.matmul(
                        psc[:, 0 : 2 * P],
                        _r(X_sb[:, k, mt * P : (mt + 1) * P], USE_R_CTX),
                        _r(Rn[:, k, :], USE_R_CTX),
                        start=(k == 0), stop=(k == ST - 1),
                    )
                # ctx_raw = w_s * Xv + Xwv   (w_s per-partition scalar)
                asm = work.tile([P, HD], F32, tag="sc512")
                nc.vector.tensor_scalar_mul(asm, psc[:, 0:HD], wc[:, mt : mt + 1])
                nc.vector.tensor_tensor(
                    ctx_sb[:, mt, hs0 : hs0 + HD], asm, psc[:, HD : 2 * HD], AOp.add
                )
                # tr partial: rowred8[:, mt] = sum_d ctx*v*ivv2
                d1 = work.tile([P, HD], F32, tag="sc512")
                nc.vector.tensor_tensor(
                    d1, ctx_sb[:, mt, hs0 : hs0 + HD], v_sb[:, mt, hs0 : hs0 + HD], AOp.mult
                )
                d2 = work.tile([P, HD], F32, tag="sc512")
                nc.vector.tensor_tensor(d2, d1, ivv2_bc[:, hs0 : hs0 + HD], AOp.mult)
                nc.vector.tensor_reduce(rowred8[:, mt : mt + 1], d2, mybir.AxisListType.X, AOp.add)
                nc.vector.tensor_scalar_add(gcol8[:, mt : mt + 1], psc[:, 2 * HD : 2 * HD + 1], 1.0)
            nc.vector.reciprocal(gcol8, gcol8)
            junk8 = work.tile([P, ST], F32, tag="junk8")
            nc.vector.tensor_tensor(junk8, rowred8, gcol8, AOp.mult)
            trsum = work.tile([P, 1], F32, tag="trsum")
            nc.vector.tensor_reduce(trsum, junk8, mybir.AxisListType.X, AOp.add)
            nc.vector.tensor_tensor(tracc, tracc, trsum, AOp.add)

            # Gram + reg partial (all at base partition 0)
            psg = psG.tile([P, P], F32, tag="g")
            for k in range(ST):
                nc.tensor.matmul(
                    psg[0:HD, 0:HD],
                    v_sb[:, k, hs0 : hs0 + HD],
                    v_sb[:, k, hs0 : hs0 + HD],
                    start=(k == 0), stop=(k == ST - 1),
                )
            g1 = work.tile([HD, HD], F32, tag="g1")
            nc.vector.tensor_copy(g1, psg[0:HD, 0:HD])
            g2 = work.tile([HD, HD], F32, tag="g2")
            nc.vector.tensor_tensor(g2, g1, g1, AOp.mult)
            g3 = work.tile([HD, HD], F32, tag="g3")
            nc.vector.tensor_scalar_mul(g3, g2, ivv2_h[:, h : h + 1])
            g4 = work.tile([HD, HD], F32, tag="g4")
            nc.vector.tensor_tensor(g4, g3, ivv2_bc[0:HD, hs0 : hs0 + HD], AOp.mult)
            regh = work.tile([HD, 1], F32, tag="regh")
            nc.vector.tensor_reduce(regh, g4, mybir.AxisListType.X, AOp.add)
            nc.vector.tensor_tensor(regacc[0:HD, :], regacc[0:HD, :], regh, AOp.add)

        # ---- transpose ctx (PE) into ctxT_pack for the Wo matmul ----
        for kg in range(KG if KSTAGE >= 3 else 0):
            for st in range(ST):
                pst = psG.tile([P, P], F32, tag="g")
                nc.tensor.transpose(pst, ctx_sb[:, st, kg * P : (kg + 1) * P], ident_sb)
                nc.vector.tensor_copy(ctxT_pack[:, kg, st * P : (st + 1) * P], pst)

        # ---- attention out: ctx @ (0.5 Wo) ----
        for st in range(ST):
            stg = work.tile([P, D], F32, tag="ostage")
            if KSTAGE >= 3:
                ps = psS.tile([P, S], F32, tag="big")
                for n2 in range(2):
                    n0, n1 = n2 * 512, min(D, (n2 + 1) * 512)
                    for k in range(KG):
                        nc.tensor.matmul(
                            ps[:, n0:n1],
                            _r(ctxT_pack[:, k, st * P : (st + 1) * P], USE_R_WO),
                            _r(woh_sb[:, k, n0:n1], USE_R_WO),
                            start=(k == 0), stop=(k == KG - 1),
                        )
                nc.vector.tensor_copy(stg, ps[:, :D])
            else:
                nc.vector.memset(stg, 0.0)
            nc.gpsimd.dma_start(att_out[st * P : (st + 1) * P, :], stg)

        red_stg = work.tile([P, 2], F32, tag="red")
        nc.vector.tensor_copy(red_stg[:, 0:1], tracc)
        nc.vector.tensor_copy(red_stg[:, 1:2], regacc)
        nc.gpsimd.dma_start(red_out, red_stg)

    nc.compile()
    return nc


_NC = None


def kernel(hidden_states, Wq, bq, Wv, bv, Wo, bo):
    global _NC
    hidden_states = np.asarray(hidden_states, np.float32)
    Wq = np.asarray(Wq, np.float32)
    bq = np.asarray(bq, np.float32)
    Wv = np.asarray(Wv, np.float32)
    bv = np.asarray(bv, np.float32)
    Wo = np.asarray(Wo, np.float32)
    bo = np.asarray(bo, np.float32)

    if _NC is None:
        _NC = build_program()

    in_maps = []
    for c in range(8):
        b, g = c // 2, c % 2
        sl = slice(g * DG, (g + 1) * DG)
        hsT_rot = np.roll(hidden_states[b].T, -g * (S // 2), axis=1)
        wq_rot = np.roll(Wq, -g * DG, axis=1)
        in_maps.append({
            "hsT": np.ascontiguousarray(hsT_rot),
            "wqf": np.ascontiguousarray(wq_rot),
            "wvh": np.ascontiguousarray(Wv[:, sl]),
            "woh": np.ascontiguousarray(0.5 * Wo[sl, :]),
            "bqc": np.ascontiguousarray(np.roll(bq, -g * DG)[:DG].reshape(KG, P).T),
            "bvr": bv[None, sl],
            "bvc": np.ascontiguousarray(bv[sl].reshape(KG, P).T),
            "ident": np.eye(P, dtype=np.float32),
        })

    res = run_bass_kernel_spmd(_NC, in_maps, core_ids=list(range(8)))
    global _LAST_RES
    _LAST_RES = res
    outs = res.results

    mixed_q = np.empty((B, S, D), np.float32)
    attention = np.empty((B, S, D), np.float32)
    tr_sum = 0.0
    reg_sum = 0.0
    for c in range(8):
        b, g = c // 2, c % 2
        # mq_out columns were computed against rotated Wq: un-rotate columns.
        # rows of mq_out = rotated s 0:512 = original rows g*512:(g+1)*512.
        mq = np.roll(outs[c]["mq_out"], g * DG, axis=1) + bq[None, :]
        mixed_q[b, g * 512 : (g + 1) * 512, :] = mq
        red = outs[c]["red_out"]
        tr_sum += float(red[:, 0].sum())
        reg_sum += float(red[:, 1].sum())
    for b in range(B):
        # att rows were computed in rotated s-order for the g=1 core
        a0 = outs[2 * b]["att_out"]
        a1 = np.roll(outs[2 * b + 1]["att_out"], S // 2, axis=0)
        attention[b] = a0 + a1 + bo[None, :]

    denom = float(HD * H * B)
    tr = np.float32(abs(denom - tr_sum) / denom)
    reg = np.float32((reg_sum - denom) / denom)
    return (attention, tr, reg, mixed_q, mixed_q)


# revision 17
# speedup vs baseline: 1.1221x; 1.1221x over previous
"""Trainium2 Bass kernel for symmetric self-attention with Laplacian regularizer.

Shapes (hardcoded): B=4, S=1024, D=768, H=12, HD=64.
Sharding: 8 cores = (batch b = c//2) x (head-half g = c%2, 6 heads each).

Key algebra (per batch, head), exploiting that scores are SYMMETRIC (k==q):
  s = q q^T / 8;  X = exp(s)  (safe without max-shift; |s| small)
  Z_i = rowsum(X); w = 1/Z;  probs = 0.5 (diag(w) X + X diag(w))
  dvec = 0.5 (1 + X w);  g = 1/(1 + X w)
  ctx_raw = diag(w)(Xv) + X(w*v);  ctx = 0.5*ctx_raw  (0.5 folded into Wo)
  V = v diag(1/vnorm) never materialized:
  tr  = |3072 - sum g_s * ctx_raw[s,d] * v[s,d] / vn2_d| / 3072
  reg = (sum_de G_de^2 / (vn2_d vn2_e) - 3072) / 3072,   G = v^T v

SPMD trick: per-core row-halves are expressed by ROTATING hsT's s-columns by
g*512 on the host (all s-dependent math is permutation-equivariant); Wq's
output columns are rotated by g*384 so qT always uses columns 0:384. Host
un-rotates the outputs.
"""

from contextlib import ExitStack

import numpy as np

import concourse.bass as bass
import concourse.bacc as bacc
import concourse.tile as tile
from concourse import mybir
from concourse.bass_utils import run_bass_kernel_spmd
import os
KSTAGE = int(os.environ.get("KSTAGE", "3"))

P = 128
B, S, D, H = 4, 1024, 768, 12
HD = 64
HPC = 6          # heads per core
DG = HPC * HD    # 384 head-dims per core
KD = D // P      # 6 contraction chunks over D
ST = S // P      # 8 sequence tiles
KG = DG // P     # 3 chunks over DG
AOp = mybir.AluOpType

F32 = mybir.dt.float32
F32R = mybir.dt.float32r

USE_R_PROJ = False
USE_R_SCORES = False
USE_R_CTX = False
USE_R_WO = False


def _r(ap, enable):
    return ap.bitcast(F32R) if enable else ap


def build_program():
    nc = bacc.Bacc("TRN2", target_bir_lowering=False)

    hsT = nc.dram_tensor("hsT", [D, S], F32, kind="ExternalInput").ap()
    wqf = nc.dram_tensor("wqf", [D, D], F32, kind="ExternalInput").ap()
    wvh = nc.dram_tensor("wvh", [D, DG], F32, kind="ExternalInput").ap()
    woh = nc.dram_tensor("woh", [DG, D], F32, kind="ExternalInput").ap()
    bqc = nc.dram_tensor("bqc", [P, KG], F32, kind="ExternalInput").ap()
    bvr = nc.dram_tensor("bvr", [1, DG], F32, kind="ExternalInput").ap()
    bvc = nc.dram_tensor("bvc", [P, KG], F32, kind="ExternalInput").ap()
    ident = nc.dram_tensor("ident", [P, P], F32, kind="ExternalInput").ap()

    mq_out = nc.dram_tensor("mq_out", [S // 2, D], F32, kind="ExternalOutput").ap()
    att_out = nc.dram_tensor("att_out", [S, D], F32, kind="ExternalOutput").ap()
    red_out = nc.dram_tensor("red_out", [P, 2], F32, kind="ExternalOutput").ap()

    def bc_dram_row(row_ap, parts):
        # DRAM row [1, N] (or [N]) replicated across `parts` partitions for DMA
        a = row_ap
        return bass.AP(tensor=a.tensor, offset=a.offset, ap=[[0, parts], list(a.ap[-1])])

    with tile.TileContext(nc) as tc, ExitStack() as ctx:
        sing = ctx.enter_context(tc.tile_pool(name="sing", bufs=1))
        work = ctx.enter_context(tc.tile_pool(name="work", bufs=2))
        dram = ctx.enter_context(tc.tile_pool(name="dram", bufs=1, space="DRAM"))
        psS = ctx.enter_context(tc.tile_pool(name="psS", bufs=2, space="PSUM"))
        psC = ctx.enter_context(tc.tile_pool(name="psC", bufs=2, space="PSUM"))
        psG = ctx.enter_context(tc.tile_pool(name="psG", bufs=1, space="PSUM"))
        psX = ctx.enter_context(tc.tile_pool(name="psX", bufs=1, space="PSUM"))

        # ---- constants / weights ----
        hsT_sb = sing.tile([P, KD, S], F32)
        nc.sync.dma_start(hsT_sb, hsT.rearrange("(c p) s -> p c s", p=P))
        # wqf shares its slot with X_sb (tag XA): wqf is dead once projections
        # finish, exactly when X is first written
        wqf_sb = sing.tile([P, KD, D], F32, tag="XA")
        nc.sync.dma_start(wqf_sb, wqf.rearrange("(c p) n -> p c n", p=P))
        wvh_sb = sing.tile([P, KD, DG], F32)
        nc.sync.dma_start(wvh_sb, wvh.rearrange("(c p) n -> p c n", p=P))
        woh_sb = sing.tile([P, KG, D], F32)
        nc.sync.dma_start(woh_sb, woh.rearrange("(c p) n -> p c n", p=P))
        bqc_sb = sing.tile([P, KG], F32)
        nc.sync.dma_start(bqc_sb, bqc)
        bvr_sb = sing.tile([P, DG], F32)
        nc.sync.dma_start(bvr_sb, bc_dram_row(bvr, P))
        bvc_sb = sing.tile([P, KG], F32)
        nc.sync.dma_start(bvc_sb, bvc)
        ident_sb = sing.tile([P, P], F32)
        nc.sync.dma_start(ident_sb, ident)

        qT_sb = sing.tile([P, KG, S], F32)
        vT_sb = sing.tile([P, KG, S], F32)
        v_sb = sing.tile([P, ST, DG], F32)
        ctx_sb = sing.tile([P, ST, DG], F32)    # ctx_raw natural [s, d]
        ctxT_pack = sing.tile([P, KG, S], F32)  # ctx_raw^T for the Wo matmul
        X_sb = sing.tile([P, ST, S], F32, tag="XA")
        vn2 = sing.tile([P, KG], F32)
        ivv2 = sing.tile([P, KG], F32)
        ivv2_bc = sing.tile([P, DG], F32)
        ivv2_h = sing.tile([HD, HPC], F32)      # per-head columnar, base partition 0
        tracc = sing.tile([P, 1], F32)
        regacc = sing.tile([P, 1], F32)
        nc.vector.memset(tracc, 0.0)
        nc.vector.memset(regacc, 0.0)

        # PE touch-matmuls: absorb each input-DMA's queue semaphore on the PE
        # one at a time (the LDWEIGHTS uop allows only a single sync wait, so
        # a real matmul must not be the first PE consumer of TWO fresh DMAs)
        touches = (hsT_sb[0:1, 0, 0:1], wqf_sb[0:1, 0, 0:1],
                   wvh_sb[0:1, 0, 0:1], woh_sb[0:1, 0, 0:1])
        pj = psG.tile([1, 1], F32, tag="g")
        for i, tch in enumerate(touches):
            nc.tensor.matmul(pj, tch, tch, start=(i == 0), stop=(i == len(touches) - 1))

        # ---- projections ----
        # mixed_q rows 0:512 of (rotated) s: out = hs @ Wq_rot (bias on host)
        for st in range(4):
            ps = psS.tile([P, S], F32, tag="big")
            for n2 in range(2):
                n0, n1 = n2 * 512, min(D, (n2 + 1) * 512)
                for k in range(KD):
                    nc.tensor.matmul(
                        ps[:, n0:n1],
                        _r(hsT_sb[:, k, st * P : (st + 1) * P], USE_R_PROJ),
                        _r(wqf_sb[:, k, n0:n1], USE_R_PROJ),
                        start=(k == 0), stop=(k == KD - 1),
                    )
            stg = work.tile([P, D], F32, tag="ostage")
            nc.vector.tensor_copy(stg, ps[:, :D])
            nc.gpsimd.dma_start(mq_out[st * P : (st + 1) * P, :], stg)

        # qT (+bq per-partition): lhsT = Wq_rot cols 0:384
        for mt in range(KG):
            ps = psS.tile([P, S], F32, tag="big")
            for n2 in range(2):
                n0, n1 = n2 * 512, (n2 + 1) * 512
                for k in range(KD):
                    nc.tensor.matmul(
                        ps[:, n0:n1],
                        _r(wqf_sb[:, k, mt * P : (mt + 1) * P], USE_R_PROJ),
                        _r(hsT_sb[:, k, n0:n1], USE_R_PROJ),
                        start=(k == 0), stop=(k == KD - 1),
                    )
            nc.vector.tensor_scalar_add(qT_sb[:, mt, :], ps, bqc_sb[:, mt : mt + 1])

        # vT (+bv per-partition)
        for mt in range(KG):
            ps = psS.tile([P, S], F32, tag="big")
            for n2 in range(2):
                n0, n1 = n2 * 512, (n2 + 1) * 512
                for k in range(KD):
                    nc.tensor.matmul(
                        ps[:, n0:n1],
                        _r(wvh_sb[:, k, mt * P : (mt + 1) * P], USE_R_PROJ),
                        _r(hsT_sb[:, k, n0:n1], USE_R_PROJ),
                        start=(k == 0), stop=(k == KD - 1),
                    )
            nc.vector.tensor_scalar_add(vT_sb[:, mt, :], ps, bvc_sb[:, mt : mt + 1])

        # v natural (+bv broadcast row)
        for st in range(ST):
            ps = psS.tile([P, S], F32, tag="big")
            for k in range(KD):
                nc.tensor.matmul(
                    ps[:, :DG],
                    _r(hsT_sb[:, k, st * P : (st + 1) * P], USE_R_PROJ),
                    _r(wvh_sb[:, k, :], USE_R_PROJ),
                    start=(k == 0), stop=(k == KD - 1),
                )
            nc.vector.tensor_tensor(v_sb[:, st, :], ps[:, :DG], bvr_sb, AOp.add)

        # vnorm^2 and reciprocals
        for mt in range(KG):
            scr = work.tile([P, S], F32, tag="scrbig")
            nc.vector.tensor_tensor(scr, vT_sb[:, mt, :], vT_sb[:, mt, :], AOp.mult)
            nc.vector.tensor_reduce(vn2[:, mt : mt + 1], scr, mybir.AxisListType.X, AOp.add)
        nc.vector.reciprocal(ivv2, vn2)

        # roundtrip ivv2 [P,3] -> flat row, broadcast to [P, DG]
        ivv2_dram = dram.tile([KG, P], F32)
        nc.gpsimd.dma_start(ivv2_dram.rearrange("t p -> p t"), ivv2)
        nc.sync.dma_start(ivv2_bc, bc_dram_row(ivv2_dram.rearrange("t p -> (t p)")[None, :], P))
        nc.sync.dma_start(ivv2_h, ivv2_dram.rearrange("t (c p) -> p (t c)", p=HD, c=2))

        # ---- per-head attention ----
        # R = [v_h | w*v_h | w | zeros] (rhs of the ctx matmul, N=256 for the
        # f32r fast path; the Xw column rides along at index 128)
        Rn = sing.tile([P, ST, 2 * P], F32)
        nc.vector.memset(Rn, 0.0)

        for h in range(HPC if KSTAGE >= 2 else 0):
            ht, hoff = h // 2, (h % 2) * HD
            hs0 = h * HD

            # scores -> X=exp(s/8) with fused rowsum Z
            Zc = work.tile([P, ST], F32, tag="zc")
            for mt in range(ST):
                ps = psS.tile([P, S], F32, tag="big")
                for n2 in range(2):
                    nc.tensor.matmul(
                        ps[:, n2 * 512 : (n2 + 1) * 512],
                        _r(qT_sb[hoff : hoff + HD, ht, mt * P : (mt + 1) * P], USE_R_SCORES),
                        _r(qT_sb[hoff : hoff + HD, ht, n2 * 512 : (n2 + 1) * 512], USE_R_SCORES),
                        start=True, stop=True,
                    )
                nc.scalar.activation(
                    X_sb[:, mt, :], ps,
                    mybir.ActivationFunctionType.Exp,
                    scale=0.125,
                    accum_out=Zc[:, mt : mt + 1],
                )

            wc = work.tile([P, ST], F32, tag="wc")
            nc.vector.reciprocal(wc, Zc)

            for st in range(ST):
                nc.vector.tensor_copy(Rn[:, st, 0:HD], v_sb[:, st, hs0 : hs0 + HD])
                nc.vector.tensor_scalar_mul(
                    Rn[:, st, HD : 2 * HD], v_sb[:, st, hs0 : hs0 + HD], wc[:, st : st + 1]
                )
                nc.vector.tensor_copy(Rn[:, st, 2 * HD : 2 * HD + 1], wc[:, st : st + 1])

            # ctx matmul (natural orientation): out[s, 0:64]=Xv, [64:128]=Xwv,
            # [128]=Xw.  lhsT = X tiles (symmetric), rhs = Rn.
            rowred8 = work.tile([P, ST], F32, tag="rowred")
            gcol8 = work.tile([P, ST], F32, tag="gcol")
            for mt in range(ST):
                psc = psC.tile([P, 512], F32, tag="ctx")
                for k in range(ST):
                    nc.tensor.matmul(
                        psc[:, 0 : 2 * P],
                        _r(X_sb[:, k, mt * P : (mt + 1) * P], USE_R_CTX),
                        _r(Rn[:, k, :], USE_R_CTX),
                        start=(k == 0), stop=(k == ST - 1),
                    )
                # ctx_raw = w_s * Xv + Xwv   (w_s per-partition scalar)
                asm = work.tile([P, HD], F32, tag="sc512")
                nc.vector.tensor_scalar_mul(asm, psc[:, 0:HD], wc[:, mt : mt + 1])
                nc.vector.tensor_tensor(
                    ctx_sb[:, mt, hs0 : hs0 + HD], asm, psc[:, HD : 2 * HD], AOp.add
                )
                # tr partial: rowred8[:, mt] = sum_d ctx*v*ivv2
                d1 = work.tile([P, HD], F32, tag="sc512")
                nc.vector.tensor_tensor(
                    d1, ctx_sb[:, mt, hs0 : hs0 + HD], v_sb[:, mt, hs0 : hs0 + HD], AOp.mult
                )
                d2 = work.tile([P, HD], F32, tag="sc512")
                nc.vector.tensor_tensor(d2, d1, ivv2_bc[:, hs0 : hs0 + HD], AOp.mult)
                nc.vector.tensor_reduce(rowred8[:, mt : mt + 1], d2, mybir.AxisListType.X, AOp.add)
                nc.vector.tensor_scalar_add(gcol8[:, mt : mt + 1], psc[:, 2 * HD : 2 * HD + 1], 1.0)
            nc.vector.reciprocal(gcol8, gcol8)
            junk8 = work.tile([P, ST], F32, tag="junk8")
            nc.vector.tensor_tensor(junk8, rowred8, gcol8, AOp.mult)
            trsum = work.tile([P, 1], F32, tag="trsum")
            nc.vector.tensor_reduce(trsum, junk8, mybir.AxisListType.X, AOp.add)
            nc.vector.tensor_tensor(tracc, tracc, trsum, AOp.add)

            # Gram + reg partial (all at base partition 0)
            psg = psG.tile([P, P], F32, tag="g")
            for k in range(ST):
                nc.tensor.matmul(
                    psg[0:HD, 0:HD],
                    v_sb[:, k, hs0 : hs0 + HD],
                    v_sb[:, k, hs0 : hs0 + HD],
                    start=(k == 0), stop=(k == ST - 1),
                )
            g1 = work.tile([HD, HD], F32, tag="g1")
            nc.vector.tensor_copy(g1, psg[0:HD, 0:HD])
            g2 = work.tile([HD, HD], F32, tag="g2")
            nc.vector.tensor_tensor(g2, g1, g1, AOp.mult)
            g3 = work.tile([HD, HD], F32, tag="g3")
            nc.vector.tensor_scalar_mul(g3, g2, ivv2_h[:, h : h + 1])
            g4 = work.tile([HD, HD], F32, tag="g4")
            nc.vector.tensor_tensor(g4, g3, ivv2_bc[0:HD, hs0 : hs0 + HD], AOp.mult)
            regh = work.tile([HD, 1], F32, tag="regh")
            nc.vector.tensor_reduce(regh, g4, mybir.AxisListType.X, AOp.add)
            nc.vector.tensor_tensor(regacc[0:HD, :], regacc[0:HD, :], regh, AOp.add)

        # ---- transpose ctx (PE) into ctxT_pack for the Wo matmul ----
        for kg in range(KG if KSTAGE >= 3 else 0):
            for st in range(ST):
                pst = psG.tile([P, P], F32, tag="g")
                nc.tensor.transpose(pst, ctx_sb[:, st, kg * P : (kg + 1) * P], ident_sb)
                nc.vector.tensor_copy(ctxT_pack[:, kg, st * P : (st + 1) * P], pst)

        # ---- attention out: ctx @ (0.5 Wo) ----
        for st in range(ST):
            stg = work.tile([P, D], F32, tag="ostage")
            if KSTAGE >= 3:
                ps = psS.tile([P, S], F32, tag="big")
                for n2 in range(2):
                    n0, n1 = n2 * 512, min(D, (n2 + 1) * 512)
                    for k in range(KG):
                        nc.tensor.matmul(
                            ps[:, n0:n1],
                            _r(ctxT_pack[:, k, st * P : (st + 1) * P], USE_R_WO),
                            _r(woh_sb[:, k, n0:n1], USE_R_WO),
                            start=(k == 0), stop=(k == KG - 1),
                        )
                nc.vector.tensor_copy(stg, ps[:, :D])
            else:
                nc.vector.memset(stg, 0.0)
            nc.gpsimd.dma_start(att_out[st * P : (st + 1) * P, :], stg)

        red_stg = work.tile([P, 2], F32, tag="red")
        nc.vector.tensor_copy(red_stg[:, 0:1], tracc)
        nc.vector.tensor_copy(red_stg[:, 1:2], regacc)
        nc.gpsimd.dma_start(red_out, red_stg)

    nc.compile()
    return nc


_EXEC = None


def _get_exec():
    """Build the Bass program once and wrap it in a cached sharded jit."""
    global _EXEC
    if _EXEC is not None:
        return _EXEC
    import jax
    import jax.numpy as jnp
    from jax.sharding import Mesh, PartitionSpec
    from jax.experimental.shard_map import shard_map
    from concourse import bass2jax as b2j
    from concourse import mybir as mb

    b2j.install_neuronx_cc_hook()
    nc = build_program()

    in_names, out_names, out_avals, zero_shapes = [], [], [], []
    partition_name = nc.partition_id_tensor.name if nc.partition_id_tensor else None
    for alloc in nc.m.functions[0].allocations:
        if not isinstance(alloc, mb.MemoryLocationSet):
            continue
        name = alloc.memorylocations[0].name
        if alloc.kind == "ExternalInput":
            if name != partition_name:
                in_names.append(name)
        elif alloc.kind == "ExternalOutput":
            out_names.append(name)
            shape = tuple(alloc.tensor_shape)
            dtype = mb.dt.np(alloc.dtype)
            out_avals.append(jax.core.ShapedArray(shape, dtype))
            zero_shapes.append((shape, dtype))
    n_params = len(in_names)
    n_outs = len(out_avals)
    all_in_names = list(in_names) + list(out_names)
    if partition_name is not None:
        all_in_names.append(partition_name)
    donate = tuple(range(n_params, n_params + n_outs))

    def _body(*args):
        operands = list(args)
        if partition_name is not None:
            operands.append(b2j.partition_id_tensor())
        outs = b2j._bass_exec_p.bind(
            *operands,
            out_avals=tuple(out_avals),
            in_names=tuple(all_in_names),
            out_names=tuple(out_names),
            lowering_input_output_aliases=(),
            sim_require_finite=True,
            sim_require_nnan=True,
            nc=nc,
        )
        return tuple(outs)

    devices = jax.devices()[:8]
    mesh = Mesh(np.asarray(devices), ("core",))
    in_specs = (PartitionSpec("core"),) * (n_params + n_outs)
    out_specs = (PartitionSpec("core"),) * n_outs
    sharded = jax.jit(
        shard_map(_body, mesh=mesh, in_specs=in_specs, out_specs=out_specs, check_rep=False),
        donate_argnums=donate, keep_unused=True,
    )
    _EXEC = (sharded, in_names, out_names, out_avals, zero_shapes)
    return _EXEC


def _make_in_maps(hidden_states, Wq, bq, Wv, bv, Wo, bo):
    in_maps = []
    for c in range(8):
        b, g = c // 2, c % 2
        sl = slice(g * DG, (g + 1) * DG)
        hsT_rot = np.roll(hidden_states[b].T, -g * (S // 2), axis=1)
        wq_rot = np.roll(Wq, -g * DG, axis=1)
        in_maps.append({
            "hsT": np.ascontiguousarray(hsT_rot),
            "wqf": np.ascontiguousarray(wq_rot),
            "wvh": np.ascontiguousarray(Wv[:, sl]),
            "woh": np.ascontiguousarray(0.5 * Wo[sl, :]),
            "bqc": np.ascontiguousarray(np.roll(bq, -g * DG)[:DG].reshape(KG, P).T),
            "bvr": np.ascontiguousarray(bv[None, sl]),
            "bvc": np.ascontiguousarray(bv[sl].reshape(KG, P).T),
            "ident": np.eye(P, dtype=np.float32),
        })
    return in_maps


def _run(in_maps):
    sharded, in_names, out_names, out_avals, zero_shapes = _get_exec()
    concat_in = [
        np.concatenate([np.asarray(in_maps[c][name]) for c in range(8)], axis=0)
        for name in in_names
    ]
    concat_zeros = [np.zeros((8 * s[0], *s[1:]), d) for (s, d) in zero_shapes]
    out_arrs = sharded(*concat_in, *concat_zeros)
    return [
        {name: np.asarray(out_arrs[i]).reshape(8, *out_avals[i].shape)[c]
         for i, name in enumerate(out_names)}
        for c in range(8)
    ]


def kernel(hidden_states, Wq, bq, Wv, bv, Wo, bo):
    hidden_states = np.asarray(hidden_states, np.float32)
    Wq = np.asarray(Wq, np.float32)
    bq = np.asarray(bq, np.float32)
    Wv = np.asarray(Wv, np.float32)
    bv = np.asarray(bv, np.float32)
    Wo = np.asarray(Wo, np.float32)
    bo = np.asarray(bo, np.float32)

    outs = _run(_make_in_maps(hidden_states, Wq, bq, Wv, bv, Wo, bo))

    mixed_q = np.empty((B, S, D), np.float32)
    attention = np.empty((B, S, D), np.float32)
    tr_sum = 0.0
    reg_sum = 0.0
    for c in range(8):
        b, g = c // 2, c % 2
        mq = np.roll(outs[c]["mq_out"], g * DG, axis=1) + bq[None, :]
        mixed_q[b, g * 512 : (g + 1) * 512, :] = mq
        red = outs[c]["red_out"]
        tr_sum += float(red[:, 0].sum())
        reg_sum += float(red[:, 1].sum())
    for b in range(B):
        a0 = outs[2 * b]["att_out"]
        a1 = np.roll(outs[2 * b + 1]["att_out"], S // 2, axis=0)
        attention[b] = a0 + a1 + bo[None, :]

    denom = float(HD * H * B)
    tr = np.float32(abs(denom - tr_sum) / denom)
    reg = np.float32((reg_sum - denom) / denom)
    return (attention, tr, reg, mixed_q, mixed_q)


# revision 34
# speedup vs baseline: 23514.3890x; 20956.5748x over previous
"""Trainium2 Bass kernel for symmetric self-attention with Laplacian regularizer.

Shapes (hardcoded): B=4, S=1024, D=768, H=12, HD=64.
Sharding: 8 cores = (batch b = c//2) x (head-half g = c%2, 6 heads each).

Key algebra (per batch, head), exploiting that scores are SYMMETRIC (k==q):
  s = q q^T / 8;  X = exp(s)  (safe without max-shift; |s| small)
  Z_i = rowsum(X); w = 1/Z;  probs = 0.5 (diag(w) X + X diag(w))
  dvec = 0.5 (1 + X w);  g = 1/(1 + X w)
  ctx_raw = diag(w)(Xv) + X(w*v);  ctx = 0.5*ctx_raw  (0.5 folded into Wo)
  V = v diag(1/vnorm) never materialized:
  tr  = |3072 - sum g_s * ctx_raw[s,d] * v[s,d] / vn2_d| / 3072
  reg = (sum_de G_de^2 / (vn2_d vn2_e) - 3072) / 3072,   G = v^T v

SPMD trick: per-core row-halves are expressed by ROTATING hsT's s-columns by
g*512 on the host (all s-dependent math is permutation-equivariant); Wq's
output columns are rotated by g*384 so qT always uses columns 0:384. Host
un-rotates the outputs.
"""

from contextlib import ExitStack

import numpy as np

import concourse.bass as bass
import concourse.bacc as bacc
import concourse.tile as tile
from concourse import mybir
from concourse.bass_utils import run_bass_kernel_spmd
import os
KSTAGE = int(os.environ.get("KSTAGE", "3"))

P = 128
B, S, D, H = 4, 1024, 768, 12
HD = 64
HPC = 6          # heads per core
DG = HPC * HD    # 384 head-dims per core
KD = D // P      # 6 contraction chunks over D
ST = S // P      # 8 sequence tiles
KG = DG // P     # 3 chunks over DG
AOp = mybir.AluOpType

F32 = mybir.dt.float32
F32R = mybir.dt.float32r

USE_R_PROJ = True
USE_R_SCORES = True
USE_R_CTX = True
USE_R_WO = True


def _r(ap, enable):
    return ap.bitcast(F32R) if enable else ap


def _w(ap, enable):
    # f32r view for WRITING (producer-side rounding required by the verifier)
    return ap.bitcast(F32R) if enable else ap


def build_program():
    nc = bacc.Bacc("TRN2", target_bir_lowering=False)

    FP = F32R if USE_R_PROJ else F32
    FW = F32R if USE_R_WO else F32
    hsT = nc.dram_tensor("hsT", [D, S], FP, kind="ExternalInput").ap()
    wqf = nc.dram_tensor("wqf", [D, D], FP, kind="ExternalInput").ap()
    wvh = nc.dram_tensor("wvh", [D, DG], FP, kind="ExternalInput").ap()
    woh = nc.dram_tensor("woh", [DG, D], FW, kind="ExternalInput").ap()
    bqc = nc.dram_tensor("bqc", [P, KG], F32, kind="ExternalInput").ap()
    bvr = nc.dram_tensor("bvr", [1, DG], F32, kind="ExternalInput").ap()
    bvc = nc.dram_tensor("bvc", [P, KG], F32, kind="ExternalInput").ap()
    ident = nc.dram_tensor("ident", [P, P], F32, kind="ExternalInput").ap()

    mq_out = nc.dram_tensor("mq_out", [S // 2, D], F32, kind="ExternalOutput").ap()
    att_out = nc.dram_tensor("att_out", [S, D], F32, kind="ExternalOutput").ap()
    red_out = nc.dram_tensor("red_out", [P, 2], F32, kind="ExternalOutput").ap()

    def bc_dram_row(row_ap, parts):
        # DRAM row [1, N] (or [N]) replicated across `parts` partitions for DMA
        a = row_ap
        return bass.AP(tensor=a.tensor, offset=a.offset, ap=[[0, parts], list(a.ap[-1])])

    with nc.allow_low_precision(reason="fp32r rounding of fp32 data, values preserved to ~19-bit mantissa"), tile.TileContext(nc) as tc, ExitStack() as ctx:
        sing = ctx.enter_context(tc.tile_pool(name="sing", bufs=1))
        work = ctx.enter_context(tc.tile_pool(name="work", bufs=2))
        dram = ctx.enter_context(tc.tile_pool(name="dram", bufs=1, space="DRAM"))
        psS = ctx.enter_context(tc.tile_pool(name="psS", bufs=2, space="PSUM"))
        psC = ctx.enter_context(tc.tile_pool(name="psC", bufs=2, space="PSUM"))
        psG = ctx.enter_context(tc.tile_pool(name="psG", bufs=2, space="PSUM"))
        psX = ctx.enter_context(tc.tile_pool(name="psX", bufs=1, space="PSUM"))

        # ---- constants / weights ----
        hsT_sb = sing.tile([P, KD, S], FP)
        hsT_r = hsT.rearrange("(c p) s -> p c s", p=P)
        for k_ in range(KD):
            nc.sync.dma_start(hsT_sb[:, k_, :], hsT_r[:, k_, :])
        # wqf shares its slot with X_sb (tag XA): wqf is dead once projections
        # finish, exactly when X is first written
        wqf_sb = sing.tile([P, KD, D], FP, tag="XA", bufs=2)
        wqf_r = wqf.rearrange("(c p) n -> p c n", p=P)
        for k_ in range(KD):
            nc.sync.dma_start(wqf_sb[:, k_, :], wqf_r[:, k_, :])
        wvh_sb = sing.tile([P, KD, DG], FP)
        nc.sync.dma_start(wvh_sb, wvh.rearrange("(c p) n -> p c n", p=P))
        woh_sb = sing.tile([P, KG, D], FW)
        nc.sync.dma_start(woh_sb, woh.rearrange("(c p) n -> p c n", p=P))
        bqc_sb = sing.tile([P, KG], F32)
        nc.sync.dma_start(bqc_sb, bqc)
        bvr_sb = sing.tile([P, DG], F32)
        nc.sync.dma_start(bvr_sb, bc_dram_row(bvr, P))
        bvc_sb = sing.tile([P, KG], F32)
        nc.sync.dma_start(bvc_sb, bvc)
        ident_sb = sing.tile([P, P], F32)
        nc.sync.dma_start(ident_sb, ident)

        qT_sb = sing.tile([P, KG, S], F32)
        vT_sb = sing.tile([P, KG, S], F32, tag="VTP")
        v_sb = sing.tile([P, ST, DG], F32)
        ctx_sb = sing.tile([P, ST, DG], F32)    # ctx_raw natural [s, d]
        ctxT_pack = sing.tile([P, KG, S], F32, tag="VTP")  # ctx_raw^T (slot shared with vT)
        vn2 = sing.tile([P, KG], F32)
        ivv2 = sing.tile([P, KG], F32)
        ivv2_bc = sing.tile([P, DG], F32)
        ivv2_h = sing.tile([HD, HPC], F32)      # per-head columnar, base partition 0
        tracc = sing.tile([P, 1], F32)
        regacc = sing.tile([P, 1], F32)
        nc.vector.memset(tracc, 0.0)
        nc.vector.memset(regacc, 0.0)

        # PE touch-matmuls: absorb each input-DMA's queue semaphore on the PE
        # one at a time (the LDWEIGHTS uop allows only a single sync wait, so
        # a real matmul must not be the first PE consumer of TWO fresh DMAs)
        touches = (hsT_sb[0:1, 0, 0:1].bitcast(F32), wqf_sb[0:1, 0, 0:1].bitcast(F32),
                   wvh_sb[0:1, 0, 0:1].bitcast(F32), woh_sb[0:1, 0, 0:1].bitcast(F32))
        pj = psG.tile([1, 1], F32, tag="g")
        for i, tch in enumerate(touches):
            nc.tensor.matmul(pj, tch, tch, start=(i == 0), stop=(i == len(touches) - 1))

        # ---- projections ----
        # mixed_q rows 0:512 of (rotated) s: out = hs @ Wq_rot (bias on host)
        for st in range(4):
            ps = psS.tile([P, S], F32, tag="big")
            for n2 in range(2):
                n0, n1 = n2 * 512, min(D, (n2 + 1) * 512)
                for k in range(KD):
                    nc.tensor.matmul(
                        ps[:, n0:n1],
                        _r(hsT_sb[:, k, st * P : (st + 1) * P], USE_R_PROJ),
                        _r(wqf_sb[:, k, n0:n1], USE_R_PROJ),
                        start=(k == 0), stop=(k == KD - 1),
                    )
            stg = work.tile([P, D], F32, tag="ostage")
            nc.scalar.copy(stg, ps[:, :D])
            nc.gpsimd.dma_start(mq_out[st * P : (st + 1) * P, :], stg)

        # qT (+bq per-partition): lhsT = Wq_rot cols 0:384
        for mt in range(KG):
            ps = psS.tile([P, S], F32, tag="big")
            for n2 in range(2):
                n0, n1 = n2 * 512, (n2 + 1) * 512
                for k in range(KD):
                    nc.tensor.matmul(
                        ps[:, n0:n1],
                        _r(wqf_sb[:, k, mt * P : (mt + 1) * P], USE_R_PROJ),
                        _r(hsT_sb[:, k, n0:n1], USE_R_PROJ),
                        start=(k == 0), stop=(k == KD - 1),
                    )
            nc.vector.tensor_scalar_add(_w(qT_sb[:, mt, :], USE_R_SCORES), ps, bqc_sb[:, mt : mt + 1])

        # vT (+bv per-partition)
        for mt in range(KG):
            ps = psS.tile([P, S], F32, tag="big")
            for n2 in range(2):
                n0, n1 = n2 * 512, (n2 + 1) * 512
                for k in range(KD):
                    nc.tensor.matmul(
                        ps[:, n0:n1],
                        _r(wvh_sb[:, k, mt * P : (mt + 1) * P], USE_R_PROJ),
                        _r(hsT_sb[:, k, n0:n1], USE_R_PROJ),
                        start=(k == 0), stop=(k == KD - 1),
                    )
            nc.vector.tensor_scalar_add(vT_sb[:, mt, :], ps, bvc_sb[:, mt : mt + 1])

        # v natural (+bv broadcast row)
        for st in range(ST):
            ps = psS.tile([P, S], F32, tag="big")
            for k in range(KD):
                nc.tensor.matmul(
                    ps[:, :DG],
                    _r(hsT_sb[:, k, st * P : (st + 1) * P], USE_R_PROJ),
                    _r(wvh_sb[:, k, :], USE_R_PROJ),
                    start=(k == 0), stop=(k == KD - 1),
                )
            nc.vector.tensor_tensor(v_sb[:, st, :], ps[:, :DG], bvr_sb, AOp.add)

        # vnorm^2 and reciprocals
        for mt in range(KG):
            scr = work.tile([P, S], F32, tag="scrbig")
            nc.vector.tensor_tensor(scr, vT_sb[:, mt, :], vT_sb[:, mt, :], AOp.mult)
            nc.vector.tensor_reduce(vn2[:, mt : mt + 1], scr, mybir.AxisListType.X, AOp.add)
        nc.vector.reciprocal(ivv2, vn2)

        # roundtrip ivv2 [P,3] -> flat row, broadcast to [P, DG]
        ivv2_dram = dram.tile([KG, P], F32)
        nc.gpsimd.dma_start(ivv2_dram.rearrange("t p -> p t"), ivv2)
        nc.sync.dma_start(ivv2_bc, bc_dram_row(ivv2_dram.rearrange("t p -> (t p)")[None, :], P))
        nc.sync.dma_start(ivv2_h, ivv2_dram.rearrange("t (c p) -> p (t c)", p=HD, c=2))

        # ---- per-head attention ----
        # R = [v_h | w*v_h | w | zeros] (rhs of the ctx matmul, N=256 for the
        # f32r fast path; the Xw column rides along at index 128)
        Rn = sing.tile([P, ST, 2 * P], F32)
        nc.vector.tensor_scalar_mul(_w(Rn, USE_R_CTX), v_sb[:, :, 0 : 2 * P], 0.0)

        for h in range(HPC if KSTAGE >= 2 else 0):
            ht, hoff = h // 2, (h % 2) * HD
            hs0 = h * HD

            # scores -> X=exp(s/8) with fused rowsum Z
            X_sb = sing.tile([P, ST, S], F32, tag="XA", bufs=2)
            Zc = work.tile([P, ST], F32, tag="zc")
            for mt in range(ST):
                ps = psS.tile([P, S], F32, tag="big")
                for n2 in range(2):
                    nc.tensor.matmul(
                        ps[:, n2 * 512 : (n2 + 1) * 512],
                        _r(qT_sb[hoff : hoff + HD, ht, mt * P : (mt + 1) * P], USE_R_SCORES),
                        _r(qT_sb[hoff : hoff + HD, ht, n2 * 512 : (n2 + 1) * 512], USE_R_SCORES),
                        start=True, stop=True,
                    )
                nc.scalar.activation(
                    _w(X_sb[:, mt, :], USE_R_CTX), ps,
                    mybir.ActivationFunctionType.Exp,
                    scale=0.125,
                    accum_out=Zc[:, mt : mt + 1],
                )

            wc = work.tile([P, ST], F32, tag="wc")
            nc.vector.reciprocal(_w(wc, USE_R_CTX), Zc)

            for st in range(ST):
                nc.vector.tensor_copy(_w(Rn[:, st, 0:HD], USE_R_CTX), v_sb[:, st, hs0 : hs0 + HD])
                nc.vector.tensor_scalar_mul(
                    _w(Rn[:, st, HD : 2 * HD], USE_R_CTX),
                    v_sb[:, st, hs0 : hs0 + HD], wc[:, st : st + 1]
                )
                nc.vector.tensor_copy(_w(Rn[:, st, 2 * HD : 2 * HD + 1], USE_R_CTX), wc[:, st : st + 1])

            # ctx matmul (natural orientation): out[s, 0:64]=Xv, [64:128]=Xwv,
            # [128]=Xw.  lhsT = X tiles (symmetric), rhs = Rn.
            rowred8 = work.tile([P, ST], F32, tag="rowred")
            gcol8 = work.tile([P, ST], F32, tag="gcol")
            for mt in range(ST):
                psc = psC.tile([P, 512], F32, tag="ctx")
                for k in range(ST):
                    nc.tensor.matmul(
                        psc[:, 0 : 2 * P],
                        _r(X_sb[:, k, mt * P : (mt + 1) * P], USE_R_CTX),
                        _r(Rn[:, k, :], USE_R_CTX),
                        start=(k == 0), stop=(k == ST - 1),
                    )
                # ctx_raw = w_s * Xv + Xwv   (w_s per-partition scalar)
                asm = work.tile([P, HD], F32, tag="sc512")
                nc.vector.tensor_scalar_mul(asm, psc[:, 0:HD], wc[:, mt : mt + 1])
                nc.vector.tensor_tensor(
                    ctx_sb[:, mt, hs0 : hs0 + HD], asm, psc[:, HD : 2 * HD], AOp.add
                )
                # tr partial: rowred8[:, mt] = sum_d ctx*v*ivv2
                d1 = work.tile([P, HD], F32, tag="sc512")
                nc.vector.tensor_tensor(
                    d1, ctx_sb[:, mt, hs0 : hs0 + HD], v_sb[:, mt, hs0 : hs0 + HD], AOp.mult
                )
                d2 = work.tile([P, HD], F32, tag="sc512")
                nc.vector.tensor_tensor(d2, d1, ivv2_bc[:, hs0 : hs0 + HD], AOp.mult)
                nc.vector.tensor_reduce(rowred8[:, mt : mt + 1], d2, mybir.AxisListType.X, AOp.add)
                nc.vector.tensor_scalar_add(gcol8[:, mt : mt + 1], psc[:, 2 * HD : 2 * HD + 1], 1.0)
            nc.vector.reciprocal(gcol8, gcol8)
            junk8 = work.tile([P, ST], F32, tag="junk8")
            nc.vector.tensor_tensor(junk8, rowred8, gcol8, AOp.mult)
            trsum = work.tile([P, 1], F32, tag="trsum")
            nc.vector.tensor_reduce(trsum, junk8, mybir.AxisListType.X, AOp.add)
            nc.vector.tensor_tensor(tracc, tracc, trsum, AOp.add)

            # Gram + reg partial (all at base partition 0)
            psg = psG.tile([P, P], F32, tag="g")
            for k in range(ST):
                nc.tensor.matmul(
                    psg[0:HD, 0:HD],
                    v_sb[:, k, hs0 : hs0 + HD],
                    v_sb[:, k, hs0 : hs0 + HD],
                    start=(k == 0), stop=(k == ST - 1),
                )
            g1 = work.tile([HD, HD], F32, tag="g1")
            nc.vector.tensor_copy(g1, psg[0:HD, 0:HD])
            g2 = work.tile([HD, HD], F32, tag="g2")
            nc.vector.tensor_tensor(g2, g1, g1, AOp.mult)
            g3 = work.tile([HD, HD], F32, tag="g3")
            nc.vector.tensor_scalar_mul(g3, g2, ivv2_h[:, h : h + 1])
            g4 = work.tile([HD, HD], F32, tag="g4")
            nc.vector.tensor_tensor(g4, g3, ivv2_bc[0:HD, hs0 : hs0 + HD], AOp.mult)
            regh = work.tile([HD, 1], F32, tag="regh")
            nc.vector.tensor_reduce(regh, g4, mybir.AxisListType.X, AOp.add)
            nc.vector.tensor_tensor(regacc[0:HD, :], regacc[0:HD, :], regh, AOp.add)

        # ---- transpose ctx (PE) into ctxT_pack for the Wo matmul ----
        for kg in range(KG if KSTAGE >= 3 else 0):
            for st in range(ST):
                pst = psG.tile([P, P], F32, tag="g")
                nc.tensor.transpose(pst, ctx_sb[:, st, kg * P : (kg + 1) * P], ident_sb)
                nc.vector.tensor_copy(_w(ctxT_pack[:, kg, st * P : (st + 1) * P], USE_R_WO), pst)

        # ---- attention out: ctx @ (0.5 Wo) ----
        for st in range(ST):
            ps = psS.tile([P, S], F32, tag="big")
            for n2 in range(2):
                n0, n1 = n2 * 512, min(D, (n2 + 1) * 512)
                for k in range(KG):
                    nc.tensor.matmul(
                        ps[:, n0:n1],
                        _r(ctxT_pack[:, k, st * P : (st + 1) * P], USE_R_WO),
                        _r(woh_sb[:, k, n0:n1], USE_R_WO),
                        start=(k == 0), stop=(k == KG - 1),
                    )
            stg = work.tile([P, D], F32, tag="ostage")
            nc.scalar.copy(stg, ps[:, :D])
            nc.gpsimd.dma_start(att_out[st * P : (st + 1) * P, :], stg)

        red_stg = work.tile([P, 2], F32, tag="red")
        nc.vector.tensor_copy(red_stg[:, 0:1], tracc)
        nc.vector.tensor_copy(red_stg[:, 1:2], regacc)
        nc.gpsimd.dma_start(red_out, red_stg)

    nc.compile()
    return nc


_EXEC = None


def _get_exec():
    """Build the Bass program once and wrap it in a cached sharded jit."""
    global _EXEC
    if _EXEC is not None:
        return _EXEC
    import jax
    import jax.numpy as jnp
    from jax.sharding import Mesh, PartitionSpec
    from jax.experimental.shard_map import shard_map
    from concourse import bass2jax as b2j
    from concourse import mybir as mb

    b2j.install_neuronx_cc_hook()
    nc = build_program()

    in_names, out_names, out_avals, zero_shapes = [], [], [], []
    partition_name = nc.partition_id_tensor.name if nc.partition_id_tensor else None
    for alloc in nc.m.functions[0].allocations:
        if not isinstance(alloc, mb.MemoryLocationSet):
            continue
        name = alloc.memorylocations[0].name
        if alloc.kind == "ExternalInput":
            if name != partition_name:
                in_names.append(name)
        elif alloc.kind == "ExternalOutput":
            out_names.append(name)
            shape = tuple(alloc.tensor_shape)
            dtype = mb.dt.np(alloc.dtype)
            out_avals.append(jax.core.ShapedArray(shape, dtype))
            zero_shapes.append((shape, dtype))
    n_params = len(in_names)
    n_outs = len(out_avals)
    all_in_names = list(in_names) + list(out_names)
    if partition_name is not None:
        all_in_names.append(partition_name)
    donate = tuple(range(n_params, n_params + n_outs))

    def _body(*args):
        operands = list(args)
        if partition_name is not None:
            operands.append(b2j.partition_id_tensor())
        outs = b2j._bass_exec_p.bind(
            *operands,
            out_avals=tuple(out_avals),
            in_names=tuple(all_in_names),
            out_names=tuple(out_names),
            lowering_input_output_aliases=(),
            sim_require_finite=True,
            sim_require_nnan=True,
            nc=nc,
        )
        return tuple(outs)

    devices = jax.devices()[:8]
    mesh = Mesh(np.asarray(devices), ("core",))
    in_specs = (PartitionSpec("core"),) * (n_params + n_outs)
    out_specs = (PartitionSpec("core"),) * n_outs
    sharded = jax.jit(
        shard_map(_body, mesh=mesh, in_specs=in_specs, out_specs=out_specs, check_rep=False),
        donate_argnums=donate, keep_unused=True,
    )
    _EXEC = (sharded, in_names, out_names, out_avals, zero_shapes)
    return _EXEC


def _make_in_maps(hidden_states, Wq, bq, Wv, bv, Wo, bo):
    in_maps = []
    for c in range(8):
        b, g = c // 2, c % 2
        sl = slice(g * DG, (g + 1) * DG)
        hsT_rot = np.roll(hidden_states[b].T, -g * (S // 2), axis=1)
        wq_rot = np.roll(Wq, -g * DG, axis=1)
        in_maps.append({
            "hsT": np.ascontiguousarray(hsT_rot),
            "wqf": np.ascontiguousarray(wq_rot),
            "wvh": np.ascontiguousarray(Wv[:, sl]),
            "woh": np.ascontiguousarray(0.5 * Wo[sl, :]),
            "bqc": np.ascontiguousarray(np.roll(bq, -g * DG)[:DG].reshape(KG, P).T),
            "bvr": np.ascontiguousarray(bv[None, sl]),
            "bvc": np.ascontiguousarray(bv[sl].reshape(KG, P).T),
            "ident": np.eye(P, dtype=np.float32),
        })
    return in_maps


def _run(in_maps):
    sharded, in_names, out_names, out_avals, zero_shapes = _get_exec()
    concat_in = [
        np.concatenate([np.asarray(in_maps[c][name]) for c in range(8)], axis=0)
        for name in in_names
    ]
    concat_zeros = [np.zeros((8 * s[0], *s[1:]), d) for (s, d) in zero_shapes]
    out_arrs = sharded(*concat_in, *concat_zeros)
    return [
        {name: np.asarray(out_arrs[i]).reshape(8, *out_avals[i].shape)[c]
         for i, name in enumerate(out_names)}
        for c in range(8)
    ]


def kernel(hidden_states, Wq, bq, Wv, bv, Wo, bo):
    hidden_states = np.asarray(hidden_states, np.float32)
    Wq = np.asarray(Wq, np.float32)
    bq = np.asarray(bq, np.float32)
    Wv = np.asarray(Wv, np.float32)
    bv = np.asarray(bv, np.float32)
    Wo = np.asarray(Wo, np.float32)
    bo = np.asarray(bo, np.float32)

    outs = _run(_make_in_maps(hidden_states, Wq, bq, Wv, bv, Wo, bo))

    mixed_q = np.empty((B, S, D), np.float32)
    attention = np.empty((B, S, D), np.float32)
    tr_sum = 0.0
    reg_sum = 0.0
    for c in range(8):
        b, g = c // 2, c % 2
        mq = np.roll(outs[c]["mq_out"], g * DG, axis=1) + bq[None, :]
        mixed_q[b, g * 512 : (g + 1) * 512, :] = mq
        red = outs[c]["red_out"]
        tr_sum += float(red[:, 0].sum())
        reg_sum += float(red[:, 1].sum())
    for b in range(B):
        a0 = outs[2 * b]["att_out"]
        a1 = np.roll(outs[2 * b + 1]["att_out"], S // 2, axis=0)
        attention[b] = a0 + a1 + bo[None, :]

    denom = float(HD * H * B)
    tr = np.float32(abs(denom - tr_sum) / denom)
    reg = np.float32((reg_sum - denom) / denom)
    return (attention, tr, reg, mixed_q, mixed_q)
